# revision 35
# baseline (speedup 1.0000x reference)
"""Causal self-attention (B=2, S=2048, D=2048, H=16) on 8 TRN2 NeuronCores.

Sharding: 2 batches x 4 head-groups.  Core c handles batch c//4 and heads
[4*(c%4) .. 4*(c%4)+3]; each core produces output columns [512*(c%4) ...].

Per core: v projection, then per local head h: causal attention over 4
q-blocks (q/k projected the previous iteration) -> two AllGathers (groups
of 4, one per token half) of the head's normalized yT (bf16) -> next head's
q/k projection.  After head 3: out-projection with heads 0-2 accumulated in
a single 12-matmul PSUM group per output tile, then head 3's pass added in
the tail (only that pass + the last AG are exposed).

vs the first working version:
  - host-side layouts let every weight land in 1-2 fat DMAs; loads are
    split across both HWDGE queues (Sync + ACT) so the serial ~650ns/issue
    DMA-issue cost doesn't gate startup (was 29us to first matmul).
  - attention uses 128-wide k-tiles with ragged q-ranges, so only the
    causal lower triangle (plus intra-tile diagonals masked via
    affine_select on GpSimd) is computed: ~15% less PE + ACT work.
  - sc PSUM tiles are [128,512] with bufs=4 for deeper sc->exp->flush
    pipelining; denominators accumulate in a [128,512] esum (no fold).

Softmax uses exp without max subtraction (logits are O(8) here);
denominators are summed across partitions with a ones-matmul and inverted
with reciprocal_approx_fast.  Compute is bf16 with fp32 PSUM accumulation;
l2 rel err vs the fp32 reference is ~6e-3.
"""

import numpy as np
import ml_dtypes

B, S, D = 2, 2048, 2048
H, HD = 16, 128
HLOC = 4           # heads per core
CW = HLOC * HD     # 512: per-core v width and out-column width
QB = 4             # q blocks of 512
DT = 16            # d tiles of 128
TB = 4             # token blocks of 512
SCALE = 1.0 / float(np.sqrt(HD))
GROUPS = [[0, 1, 2, 3], [4, 5, 6, 7]]

_cache = {}


def _build():
    import concourse.tile as tile
    import concourse.mybir as mybir
    from concourse import bacc

    BF = mybir.dt.bfloat16
    F32 = mybir.dt.float32

    nc = bacc.Bacc("TRN2", target_bir_lowering=False, debug=False, num_devices=8)

    # Inputs (per-core shards, host-prepared; layouts chosen for fat DMAs)
    xT = nc.dram_tensor("xT", [D, S], BF, kind="ExternalInput")          # x[batch].T
    # wqk[h][qk][:, dt*128+m] = w_qkv[dt*128+p, qk*D + (4g+h)*128 + m]
    wqk = nc.dram_tensor("wqk", [HLOC, 2, 128, DT * 128], BF, kind="ExternalInput")
    # wv[p, dt*512+c] = w_qkv[dt*128+p, 2D + cols]
    wv = nc.dram_tensor("wv", [128, DT * CW], BF, kind="ExternalInput")
    # bqk[p, 2h+qk] = b_qkv[qk*D + (4g+h)*128 + p]
    bqk = nc.dram_tensor("bqk", [128, 2 * HLOC], F32, kind="ExternalInput")
    # bb = [b_out[cols] | b_qkv[2D:][cols]]
    bb = nc.dram_tensor("bb", [1, 2 * CW], F32, kind="ExternalInput")
    # wout[h][p, i*512+c] = w_out[512*i + 128*h + p, cols]
    wout = nc.dram_tensor("wout", [HLOC, 128, 4 * CW], BF, kind="ExternalInput")
    out = nc.dram_tensor("out", [S, CW], F32, kind="ExternalOutput")

    # per (head, token-half) AG buffers; half 1 = tokens 1024:2048
    # (q-blocks 3,2 -- computed first), half 0 = tokens 0:1024
    ag_in = {(h, hf): nc.dram_tensor(f"ag_in{h}_{hf}", [128, 1024], BF,
                                     kind="Internal")
             for h in range(HLOC) for hf in range(2)}
    ag_out = {(h, hf): nc.dram_tensor(f"ag_out{h}_{hf}", [512, 1024], BF,
                                      kind="Internal")
              for h in range(HLOC) for hf in range(2)}

    with tile.TileContext(nc) as tc:
        with (
            tc.tile_pool(name="const", bufs=1) as constp,
            tc.tile_pool(name="pers", bufs=1) as pers,
            tc.tile_pool(name="work", bufs=2) as work,
            tc.tile_pool(name="psum", bufs=2, space="PSUM") as psum,
        ):
            # ---- constants ----
            ones = constp.tile([128, 1], BF, name="ones")
            nc.gpsimd.memset(ones[:], 1.0)

            # ---- loads. Transient data lives in one rotating pool of
            # [128,1024] bf16 tiles ("p2k", 40 slots): wv (8 tiles, dead
            # right after v-proj) is recycled by the first ygt tiles; x
            # (32 tiles, dead once qk(3) consumes it) by the rest.
            # First-matmul critical set is wv + xt(tb0): wv rides ACT's
            # HWDGE queue, the tb0 halves ride Sync.
            wv_t = []
            for q in range(8):
                t = work.tile([128, 1024], BF, name=f"wv{q}", tag="p2k",
                              bufs=40)
                nc.scalar.dma_start(t[:], wv[:, q * 1024:(q + 1) * 1024])
                wv_t.append(t)
            xt01, xt23 = {}, {}
            for dt in range(DT):
                t = work.tile([128, 1024], BF, name=f"xt01_{dt}", tag="p2k",
                              bufs=40)
                nc.sync.dma_start(t[:, 0:512],
                                  xT[dt * 128:(dt + 1) * 128, 0:512])
                xt01[dt] = t
            wqk_sb = {}

            def load_wqk(h):
                for qk in range(2):
                    t = work.tile([128, DT * 128], BF, name=f"wqk{h}{qk}",
                                  tag="wqk", bufs=4)
                    nc.scalar.dma_start(t[:], wqk[h, qk])
                    wqk_sb[(h, qk)] = t

            load_wqk(0)
            bqk_sb = constp.tile([128, 2 * HLOC], F32, name="bqk_sb")
            nc.scalar.dma_start(bqk_sb[:], bqk[:])
            bb_sb = constp.tile([1, 2 * CW], F32, name="bb_sb")
            nc.scalar.dma_start(bb_sb[:], bb[:])
            bias_bc = constp.tile([128, CW], F32, name="bias_bc")
            nc.gpsimd.partition_broadcast(bias_bc[:], bb_sb[:, 0:CW],
                                          channels=128)
            vbias_bc = constp.tile([128, CW], F32, name="vbias_bc")
            nc.gpsimd.partition_broadcast(vbias_bc[:], bb_sb[:, CW:2 * CW],
                                          channels=128)
            for dt in range(DT):
                nc.scalar.dma_start(xt01[dt][:, 512:1024],
                                    xT[dt * 128:(dt + 1) * 128, 512:1024])
            for dt in range(DT):
                t = work.tile([128, 1024], BF, name=f"xt23_{dt}", tag="p2k",
                              bufs=40)
                eng = nc.sync if dt % 2 else nc.scalar
                eng.dma_start(t[:], xT[dt * 128:(dt + 1) * 128, 1024:2048])
                xt23[dt] = t

            def xt_ap(dt, tb):
                """xT tile [128, 512] for (dt block, token block)."""
                if tb < 2:
                    return xt01[dt][:, tb * 512:(tb + 1) * 512]
                return xt23[dt][:, (tb - 2) * 512:(tb - 1) * 512]

            wout_sb = {}

            def load_wout():
                for h in range(HLOC):
                    t = work.tile([128, 4 * CW], BF, name=f"wout{h}",
                                  tag="wout", bufs=HLOC)
                    nc.scalar.dma_start(t[:], wout[h])
                    wout_sb[h] = t

            # ---- persistent tiles ----
            vt = [pers.tile([128, CW], BF, name=f"v{t}", tag=f"v{t}")
                  for t in range(16)]
            part = [pers.tile([128, CW], BF, name=f"part{t}", tag=f"part{t}")
                    for t in range(16)]

            # ---- projections ----
            def v_proj_tb(tb):
                for t in range(4 * tb, 4 * tb + 4):
                    j = t % 4
                    acc = psum.tile([128, CW], F32, name="acc_v", tag="acc",
                                    bufs=2)
                    for dt in range(DT):
                        nc.tensor.matmul(
                            acc[:],
                            xt_ap(dt, tb)[:, j * 128:(j + 1) * 128],
                            wv_t[dt // 2][:, (dt % 2) * CW:(dt % 2 + 1) * CW],
                            start=(dt == 0), stop=(dt == DT - 1),
                        )
                    nc.vector.tensor_tensor(vt[t][:], acc[:], vbias_bc[:],
                                            mybir.AluOpType.add)

            def qk_proj_tb(h, dests, tb):
                for qk in range(2):
                    w = wqk_sb[(h, qk)]
                    acc = psum.tile([128, 512], F32, name="acc_qk",
                                    tag="acc", bufs=2)
                    for dt in range(DT):
                        nc.tensor.matmul(
                            acc[:], w[:, dt * 128:(dt + 1) * 128],
                            xt_ap(dt, tb),
                            start=(dt == 0), stop=(dt == DT - 1),
                        )
                    # DVE, not ACT: keeps the projection epilogue off the
                    # exp engine and out of the startup DMA-issue jam
                    nc.vector.tensor_scalar_add(
                        dests[qk][:, tb * 512:(tb + 1) * 512], acc[:],
                        bqk_sb[:, 2 * h + qk:2 * h + qk + 1],
                    )

            def qk_dests(h):
                return tuple(work.tile([128, S], BF, name=f"qkT_{h}_{qk}",
                                       tag="qkT", bufs=4)
                             for qk in range(2))

            def qk_proj(h):
                dests = qk_dests(h)
                for tb in range(TB):
                    qk_proj_tb(h, dests, tb)
                return dests

            # ---- attention for one head (q-blocks descending) + its AG ----
            # Per q-block: k-tiles 0..4qb+3; the last four (diagonal) have
            # ragged valid q-ranges [128j, 512) and an intra-tile triangle
            # masked in-place on GpSimd.
            def attention_head(h, qTh, kTh, hooks={}):
                for qb in (3, 2, 1, 0):
                    nk = 4 * qb + 4
                    y_ps = psum.tile([128, 512], F32, name="y_ps", tag="y")
                    esum = work.tile([128, 512], BF, name="esum", tag="esum",
                                     bufs=2)
                    for kt in range(nk):
                        j = kt - 4 * qb
                        off = 128 * j if j > 0 else 0
                        L = 512 - off
                        sc = psum.tile([128, 512], F32, name="sc", tag="s",
                                       bufs=4)
                        nc.tensor.matmul(
                            sc[:, 0:L],
                            kTh[:, kt * 128:(kt + 1) * 128],
                            qTh[:, qb * 512 + off:(qb + 1) * 512],
                            start=True, stop=True,
                        )
                        e = work.tile([128, 512], BF, name="expT", tag="expT",
                                      bufs=6)
                        nc.scalar.activation(
                            e[:, 0:L], sc[:, 0:L],
                            mybir.ActivationFunctionType.Exp, scale=SCALE,
                        )
                        if j >= 0:
                            # keep where q - k >= 0 within the leading 128
                            nc.gpsimd.affine_select(
                                out=e[:, 0:128], in_=e[:, 0:128],
                                compare_op=mybir.AluOpType.is_ge, fill=0.0,
                                base=0, channel_multiplier=-1,
                                pattern=[[1, 128]],
                            )
                        if kt == 0:
                            nc.vector.tensor_copy(esum[:], e[:])
                        else:
                            nc.vector.tensor_tensor(
                                esum[:, off:512], esum[:, off:512], e[:, 0:L],
                                mybir.AluOpType.add)
                        nc.tensor.matmul(
                            y_ps[:, off:512],
                            vt[kt][:, h * 128:(h + 1) * 128],
                            e[:, 0:L],
                            start=(kt == 0), stop=(kt == nk - 1),
                        )

                    sum_ps = psum.tile([1, 512], F32, name="sum_ps", tag="y")
                    nc.tensor.matmul(sum_ps[:], ones[:], esum[:],
                                     start=True, stop=True)
                    recip = work.tile([1, 512], F32, name="recip", tag="recip",
                                      bufs=2)
                    nc.vector.reciprocal_approx_fast(recip[:], sum_ps[:])
                    rbc = work.tile([128, 512], F32, name="rbc", tag="rbc",
                                    bufs=2)
                    nc.gpsimd.partition_broadcast(rbc[:], recip[:], channels=128)
                    ynorm = work.tile([128, 512], BF, name="ynorm", tag="ynorm",
                                      bufs=3)
                    nc.vector.tensor_tensor(ynorm[:], y_ps[:], rbc[:],
                                            mybir.AluOpType.mult)
                    hf, co = qb // 2, (qb % 2) * 512
                    nc.sync.dma_start(
                        ag_in[(h, hf)][:, co:co + 512], ynorm[:])
                    if qb in (2, 0):
                        nc.gpsimd.collective_compute(
                            "AllGather", mybir.AluOpType.bypass,
                            replica_groups=GROUPS,
                            ins=[ag_in[(h, hf)].ap()],
                            outs=[ag_out[(h, hf)].ap()],
                        )
                    if qb in hooks:
                        hooks[qb]()

            # ---- out-projection ----
            # ygt[(h, hf, i)] = ag_out[(h, hf)] rank-row block i, both halves
            ygt = {}

            def load_ygt(h, hf, scalar_only=False):
                # p2k slots: the first 8 loads recycle wv's tiles, the rest
                # recycle x tiles; emission points are chosen so the AG
                # being read is already complete (no head-of-line blocking)
                for i in range(4):
                    t = work.tile([128, 1024], BF, name=f"yg_{h}_{hf}_{i}",
                                  tag="p2k", bufs=40)
                    eng = nc.scalar if (scalar_only or i % 2) else nc.sync
                    eng.dma_start(
                        t[:], ag_out[(h, hf)][i * 128:(i + 1) * 128, :])
                    ygt[(h, hf, i)] = t

            def outproj_pass3(hf):
                for co in (0, 512):
                    tc_ = 2 * hf + co // 512
                    for j in range(4):
                        t = tc_ * 4 + j
                        acc = psum.tile([128, CW], F32, name="acc_o",
                                        tag="acc", bufs=2)
                        for i in range(4):
                            nc.tensor.matmul(
                                acc[:],
                                ygt[(3, hf, i)][:, co + j * 128:
                                                co + (j + 1) * 128],
                                wout_sb[3][:, i * CW:(i + 1) * CW],
                                start=(i == 0), stop=(i == 3),
                            )
                        osb = work.tile([128, CW], F32, name="osb",
                                        tag="osb", bufs=3)
                        nc.vector.tensor_tensor(osb[:], part[t][:], acc[:],
                                                mybir.AluOpType.add)
                        nc.sync.dma_start(
                            out[t * 128:(t + 1) * 128, :], osb[:])

            def outproj_pass012(hf):
                    for co in (0, 512):
                        tc_ = 2 * hf + co // 512
                        for j in range(4):
                            t = tc_ * 4 + j
                            acc = psum.tile([128, CW], F32, name="acc_o",
                                            tag="acc", bufs=2)
                            for h in range(3):
                                for i in range(4):
                                    nc.tensor.matmul(
                                        acc[:],
                                        ygt[(h, hf, i)][:, co + j * 128:
                                                        co + (j + 1) * 128],
                                        wout_sb[h][:, i * CW:(i + 1) * CW],
                                        start=(h == 0 and i == 0),
                                        stop=(h == 2 and i == 3),
                                    )
                            nc.vector.tensor_tensor(part[t][:], acc[:],
                                                    bias_bc[:],
                                                    mybir.AluOpType.add)

            # ---- head pipeline; v-proj and qk(0) interleave per token
            # block so neither waits on the tb>0 x loads ----
            qk_tiles = qk_dests(0)
            for tb in range(TB):
                v_proj_tb(tb)
                qk_proj_tb(0, qk_tiles, tb)
            # ygt prefetch points: head h's gathers load during att(h+2) --
            # two full heads after the AG trigger, so even the skew-absorbing
            # first AG is long complete and the issue ops never block their
            # queues.  (0,*) lands in the dead wv slots; later ones recycle
            # x tiles, dead once qk(3) has consumed them (att(3) start).
            hooks = {
                2: {3: lambda: load_ygt(0, 1), 2: lambda: load_ygt(0, 0)},
                3: {3: lambda: load_ygt(1, 1), 2: lambda: load_ygt(1, 0),
                    1: lambda: load_ygt(2, 1)},
            }
            for h in range(HLOC):
                if h + 1 < HLOC:
                    load_wqk(h + 1)
                attention_head(h, *qk_tiles, hooks=hooks.get(h, {}))
                if h == 0:
                    load_wout()
                qk_tiles = qk_proj(h + 1) if h + 1 < HLOC else None
            # the remaining gathers land at the very end; their loads ride
            # the ACT queue, which has nothing compute-critical left
            load_ygt(2, 0, scalar_only=True)
            load_ygt(3, 1, scalar_only=True)
            outproj_pass012(1)
            load_ygt(3, 0, scalar_only=True)
            outproj_pass3(1)
            outproj_pass012(0)
            outproj_pass3(0)

    nc.compile()
    return nc


def _prep_inputs(x, w_qkv, b_qkv, w_out, b_out):
    """Host-side sharding/layout. Returns in_maps for the 8 cores."""
    bf16 = ml_dtypes.bfloat16
    x = np.asarray(x, dtype=np.float32)
    w_qkv = np.asarray(w_qkv, dtype=np.float32)
    b_qkv = np.asarray(b_qkv, dtype=np.float32)
    w_out = np.asarray(w_out, dtype=np.float32)
    b_out = np.asarray(b_out, dtype=np.float32)

    xT_b = [np.ascontiguousarray(x[b].T).astype(bf16) for b in range(B)]

    in_maps = []
    for c in range(8):
        b, g = c // 4, c % 4
        cols = slice(CW * g, CW * (g + 1))

        # wqk[h][qk] = [128, dt*128+m] (dt-major along free)
        wqk = np.empty((HLOC, 2, 128, DT * 128), np.float32)
        bqk = np.empty((128, 2 * HLOC), np.float32)
        for h in range(HLOC):
            gh = 4 * g + h
            for qk in range(2):
                wcol = w_qkv[:, qk * D + 128 * gh: qk * D + 128 * (gh + 1)]
                # [2048, 128] -> [dt, p, m] -> [p, dt, m]
                wqk[h, qk] = wcol.reshape(DT, 128, 128).transpose(1, 0, 2) \
                                 .reshape(128, DT * 128)
                bqk[:, 2 * h + qk] = b_qkv[qk * D + 128 * gh:
                                           qk * D + 128 * (gh + 1)]

        # wv = [128, dt*512+c]
        wv_ = w_qkv[:, 2 * D:3 * D][:, cols]
        wv_m = wv_.reshape(DT, 128, CW).transpose(1, 0, 2).reshape(128, DT * CW)

        # w_out rows permuted to the AG's rank-major row order per head chunk
        wout_loc = w_out[:, cols]
        wout_t = np.empty((HLOC, 128, 4 * CW), np.float32)
        for h in range(HLOC):
            for i in range(4):
                wout_t[h, :, i * CW:(i + 1) * CW] = \
                    wout_loc[512 * i + 128 * h: 512 * i + 128 * (h + 1), :]

        bb = np.concatenate([b_out[cols], b_qkv[2 * D:3 * D][cols]])

        in_maps.append({
            "xT": xT_b[b],
            "wqk": np.ascontiguousarray(wqk).astype(bf16),
            "wv": np.ascontiguousarray(wv_m).astype(bf16),
            "bqk": np.ascontiguousarray(bqk),
            "bb": np.ascontiguousarray(bb.reshape(1, 2 * CW)),
            "wout": np.ascontiguousarray(wout_t).astype(bf16),
        })
    return in_maps


def kernel(x, w_qkv, b_qkv, w_out, b_out, _trace=False, _trace_kwargs=None):
    from concourse.bass_utils import run_bass_kernel_spmd

    if "nc" not in _cache:
        _cache["nc"] = _build()
    nc = _cache["nc"]

    in_maps = _prep_inputs(x, w_qkv, b_qkv, w_out, b_out)
    res = run_bass_kernel_spmd(
        nc, in_maps, core_ids=list(range(8)),
        trace=_trace, **(_trace_kwargs or {}),
    )

    out = np.empty((B, S, D), dtype=np.float32)
    for c in range(8):
        b, g = c // 4, c % 4
        out[b][:, CW * g:CW * (g + 1)] = res.results[c]["out"]
    kernel.last_result = res
    return out
